# revision 1
# baseline (speedup 1.0000x reference)
"""MoE routing kernel for Trainium2 (8 NeuronCores, expert-parallel).

out[i] = x[i] + relu(x[i] @ W[e].T + b[e]),  e = cam_pred_ids[i]

Strategy: route tokens by expert on the host (the sharding step), so core e
computes ONLY expert e's tokens with ONLY W[e] (16MB instead of 128MB).
All device-side layouts are pre-transposed on the host so every DMA is
contiguous: the device computes hT[o, n] = sum_k WT[k, o] * xT[k, n] with
K on partitions for both operands, then outT = xT + relu(hT + b) and the
host transposes/scatters back.
"""

import os
import numpy as np

import concourse.bass as bass
from concourse import bacc
import concourse.mybir as mybir
import concourse.tile as tile
from concourse.bass_utils import run_bass_kernel_spmd

NUM_EXPERTS = 8
DIM = 2048
KT = DIM // 128  # 16 k-tiles
OT = DIM // 128  # 16 o-tiles

# matmul operand dtype mode: 'f16' (default: 1 cyc/row, fully-overlapped
# weight loads, end-to-end rel err ~1.1e-4), 'f32r' (fp32 storage,
# TF32-like matmul, 1 cyc/row but exposed fp32 weight loads), 'f32'
# (exact, 4 cyc/row), 'bf16'. The residual add always uses exact fp32 x.
MODE = os.environ.get("BASS_MOE_MODE", "f16")


def _chunks(np_tokens: int) -> list[tuple[int, int]]:
    """Split the free dim into matmul chunks of <=512, each >=256 when
    possible (float32r runs 4x slower below 256 moving columns)."""
    out = []
    pos = 0
    rem = np_tokens
    while rem > 0:
        if rem > 512 + 256:
            take = 512
        elif rem > 512:
            take = (rem + 1) // 2  # two chunks, both >=256
        else:
            take = rem
        out.append((pos, take))
        pos += take
        rem -= take
    return out


def _build_nc(np_tokens: int, mode: str):
    f32 = mybir.dt.float32
    mm_dt = {
        "f32r": mybir.dt.float32r,
        "f32": mybir.dt.float32,
        "f16": mybir.dt.float16,
        "bf16": mybir.dt.bfloat16,
    }[mode]
    sixteen_bit = mode in ("f16", "bf16")

    # For f32/f32r the x input doubles as matmul rhs and residual; walrus
    # requires fp32r matmul operands to be produced as fp32r, so the tiles
    # carry mm_dt and get bitcast to f32 for the residual add (same bits).
    xt_dt = mm_dt if mode in ("f32", "f32r") else f32

    nc = bacc.Bacc()
    wt_d = nc.declare_dram_parameter("wt", [OT, 128, KT, 128], mm_dt, isOutput=False)
    xt_d = nc.declare_dram_parameter("xt", [DIM, np_tokens], xt_dt, isOutput=False)
    if sixteen_bit:
        xtm_d = nc.declare_dram_parameter(
            "xtm", [DIM, np_tokens], mm_dt, isOutput=False
        )
    b_d = nc.declare_dram_parameter("b", [128, OT], f32, isOutput=False)
    out_d = nc.declare_dram_parameter("out", [DIM, np_tokens], f32, isOutput=True)

    chunks = _chunks(np_tokens)
    relu = mybir.ActivationFunctionType.Relu

    with tile.TileContext(nc) as tc:
        with (
            tc.tile_pool(name="xp", bufs=1) as xp,
            tc.tile_pool(name="wp", bufs=4) as wp,
            tc.tile_pool(name="op", bufs=3) as op,
            tc.tile_pool(name="bp", bufs=1) as bp,
            tc.tile_pool(name="pp", bufs=2, space="PSUM") as pp,
        ):
            # First weight tile is DMA'd BEFORE the x loads (split in pieces
            # so the first matmul group can start on kt=0 early): queue
            # semaphore targets accumulate in program order, so anything
            # queued ahead delays the first matmul's wait from clearing.
            # DMA triggers cost ~650ns of serial sequencer time each
            # (DIRECT2D) and each HWDGE ring drains serially, so split by
            # role: sync ring carries the weight stream (+ half the matmul
            # x), scalar ring carries bias + other half + the f32 residual
            # copy + output stores. The PE-gating loads (wt, xmm) never
            # queue behind the residual load.
            wtiles = {}
            wtiles[0] = wp.tile([128, KT, 128], mm_dt, name="wtile", tag="w")
            for q in range(2):
                nc.sync.dma_start(
                    out=wtiles[0][:, q * 8 : (q + 1) * 8, :],
                    in_=wt_d[0, :, q * 8 : (q + 1) * 8, :],
                )

            btile = bp.tile([128, OT], f32, name="btile")
            nc.scalar.dma_start(out=btile, in_=b_d[:, :])

            xt_r = xt_d.rearrange("(t p) n -> p t n", p=128)
            xall = xp.tile([128, KT, np_tokens], xt_dt, name="xall")
            # x loads as 8 slabs of 2 k-tiles each, alternating rings
            slabs = [(s, 2) for s in range(0, KT, 2)]
            if sixteen_bit:
                xtm_r = xtm_d.rearrange("(t p) n -> p t n", p=128)
                xmall = xp.tile([128, KT, np_tokens], mm_dt, name="xmall")
                for q, (s0, w) in enumerate(slabs):
                    eng = nc.sync if q % 2 == 0 else nc.scalar
                    eng.dma_start(
                        out=xmall[:, s0 : s0 + w, :],
                        in_=xtm_r[:, s0 : s0 + w, :],
                    )
                xmm = [xmall[:, kt, :] for kt in range(KT)]
            else:
                for q, (s0, w) in enumerate(slabs):
                    eng = nc.sync if q % 2 == 0 else nc.scalar
                    eng.dma_start(
                        out=xall[:, s0 : s0 + w, :],
                        in_=xt_r[:, s0 : s0 + w, :],
                    )

            # Whole weight stream up front on the sync ring; slot waits
            # (wp bufs) pace it automatically behind the matmul progress.
            for ot in range(1, OT):
                wtiles[ot] = wp.tile([128, KT, 128], mm_dt, name="wtile", tag="w")
                nc.sync.dma_start(out=wtiles[ot], in_=wt_d[ot])

            if sixteen_bit:
                # residual copy: only needed when psum groups close — load
                # late, on the scalar ring
                for q in range(8):
                    nc.scalar.dma_start(
                        out=xall[:, q * 2 : (q + 1) * 2, :],
                        in_=xt_r[:, q * 2 : (q + 1) * 2, :],
                    )
            xall_f32 = xall.bitcast(f32) if mode == "f32r" else xall
            xres = [xall_f32[:, kt, :] for kt in range(KT)]
            if not sixteen_bit:
                xmm = [xall[:, kt, :] for kt in range(KT)]

            for ot in range(OT):
                wtile = wtiles[ot]
                otile = op.tile([128, np_tokens], f32, name="otile", tag="o")
                psums = [
                    pp.tile([128, ch], f32, name=f"ps{ci}", tag=f"ps{ci}")
                    for ci, (_, ch) in enumerate(chunks)
                ]
                for kt in range(KT):
                    lhsT = wtile[:, kt, :]
                    for ci, (n0, ch) in enumerate(chunks):
                        nc.tensor.matmul(
                            psums[ci],
                            lhsT,
                            xmm[kt][:, n0 : n0 + ch],
                            start=(kt == 0),
                            stop=(kt == KT - 1),
                        )
                for ci, (n0, ch) in enumerate(chunks):
                    nc.scalar.activation(
                        otile[:, n0 : n0 + ch],
                        psums[ci],
                        relu,
                        bias=btile[:, ot : ot + 1],
                    )
                    nc.vector.tensor_add(
                        otile[:, n0 : n0 + ch],
                        otile[:, n0 : n0 + ch],
                        xres[ot][:, n0 : n0 + ch],
                    )
                    if ot == OT - 1:
                        # last tile: store per-chunk to overlap the tail
                        nc.scalar.dma_start(
                            out=out_d[ot * 128 : (ot + 1) * 128, n0 : n0 + ch],
                            in_=otile[:, n0 : n0 + ch],
                        )
                if ot < OT - 1:
                    nc.scalar.dma_start(
                        out=out_d[ot * 128 : (ot + 1) * 128, :], in_=otile
                    )
    nc.compile()
    return nc


def kernel(x, cam_pred_ids, W, b, _want_results=False):
    x = np.ascontiguousarray(np.asarray(x), dtype=np.float32)
    W = np.asarray(W, dtype=np.float32)
    b = np.asarray(b, dtype=np.float32)
    ids = np.asarray(cam_pred_ids).astype(np.int64)
    batch = x.shape[0]

    counts = np.bincount(ids, minlength=NUM_EXPERTS)
    order = np.argsort(ids, kind="stable")
    np_tokens = max(512, int(counts.max()))

    # per-expert padded token index lists (pad with token 0; discarded later)
    starts = np.zeros(NUM_EXPERTS + 1, dtype=np.int64)
    np.cumsum(counts, out=starts[1:])
    idx = np.zeros((NUM_EXPERTS, np_tokens), dtype=np.int64)
    for e in range(NUM_EXPERTS):
        idx[e, : counts[e]] = order[starts[e] : starts[e + 1]]

    mode = MODE
    mm_np = {
        "f32r": np.float32,
        "f32": np.float32,
        "f16": np.float16,
        "bf16": None,  # ml_dtypes.bfloat16, resolved lazily
    }[mode]
    if mode == "bf16":
        import ml_dtypes

        mm_np = ml_dtypes.bfloat16
    sixteen_bit = mode in ("f16", "bf16")

    in_maps = []
    for e in range(NUM_EXPERTS):
        xg = x[idx[e]]  # [Np, DIM]
        xt = np.ascontiguousarray(xg.T)  # [DIM, Np]
        # wdev[ot, k, kt, o] = W[e][ot*128+o, kt*128+k]
        wdev = np.ascontiguousarray(
            W[e].reshape(OT, 128, KT, 128).transpose(0, 3, 2, 1), dtype=mm_np
        )
        m = {
            "wt": wdev,
            "xt": xt,
            "b": np.ascontiguousarray(b[e].reshape(OT, 128).T),
        }
        if sixteen_bit:
            m["xtm"] = np.ascontiguousarray(xt, dtype=mm_np)
        in_maps.append(m)

    nc = _build_nc(np_tokens, mode)
    res = run_bass_kernel_spmd(
        nc,
        in_maps,
        core_ids=list(range(NUM_EXPERTS)),
        trace=bool(int(os.environ.get("BASS_MOE_TRACE", "0"))),
    )

    out = np.empty_like(x)
    for e in range(NUM_EXPERTS):
        oute = res.results[e]["out"]  # [DIM, Np]
        valid = idx[e, : counts[e]]
        out[valid] = oute.T[: counts[e]]
    if _want_results:
        return out, res
    return out



# revision 4
# speedup vs baseline: 1.7541x; 1.7541x over previous
"""MoE routing kernel for Trainium2 (8 NeuronCores, expert-parallel, fp8).

out[i] = x[i] + relu(x[i] @ W[e].T + b[e]),  e = cam_pred_ids[i]

Strategy: route tokens by expert on the host, so core e computes ONLY
expert e's tokens with ONLY W[e]. The matmul runs in fp8 e4m3 with
DoubleRow perf mode (two 128-deep K planes contracted per instruction),
with W pre-scaled by 32 on the host to stay in e4m3 normal range; the
activation step rescales by 1/32 and applies bias+relu. The residual is
added from a separate f16 copy of x and the output is stored as f16
(quantization error is dominated by the fp8 weights; measured end-to-end
rel err ~1.4e-2 on the reference distribution, within the 2e-2 budget).

All device-side layouts are pre-transposed on the host so every DMA
moves >=2KB contiguous per partition.
"""

import os
import numpy as np
import ml_dtypes

import concourse.bass as bass
from concourse import bacc
import concourse.mybir as mybir
import concourse.tile as tile
from concourse.bass_utils import run_bass_kernel_spmd

NUM_EXPERTS = 8
DIM = 2048
KT2 = DIM // 256  # 8 double-row k groups (256 contraction each)
OT = DIM // 128  # 16 o-tiles

W_SCALE = 32.0
MODE = "fp8dr"  # fp8 e4m3 DoubleRow


def _chunks(np_tokens: int) -> list[tuple[int, int]]:
    """Split the free dim into matmul chunks of <=512 (PSUM bank limit),
    each >=256 when possible."""
    out = []
    pos = 0
    rem = np_tokens
    while rem > 0:
        if rem > 512 + 256:
            take = 512
        elif rem > 512:
            take = (rem + 1) // 2  # two chunks, both >=256
        else:
            take = rem
        out.append((pos, take))
        pos += take
        rem -= take
    return out


def _build_nc(np_tokens: int):
    f32 = mybir.dt.float32
    f16 = mybir.dt.float16
    f8 = mybir.dt.float8e4

    nc = bacc.Bacc()
    # wt[ot, k_lo, kt2, i, o] = 32*W[e][128*ot+o, 256*kt2+128*i+k_lo]
    wt_d = nc.declare_dram_parameter("wt", [OT, 128, KT2, 2, 128], f8, isOutput=False)
    # x8[k_lo, kt2, i, n] = fp8(x[n, 256*kt2+128*i+k_lo])
    x8_d = nc.declare_dram_parameter("x8", [128, KT2, 2, np_tokens], f8, isOutput=False)
    # xr[(dt p), n] = f16(x[n, 128*dt+p])   (residual, d on partitions)
    xr_d = nc.declare_dram_parameter("xr", [DIM, np_tokens], f16, isOutput=False)
    b_d = nc.declare_dram_parameter("b", [128, OT], f32, isOutput=False)
    out_d = nc.declare_dram_parameter("out", [OT, 128, np_tokens], f16, isOutput=True)

    chunks = _chunks(np_tokens)
    relu = mybir.ActivationFunctionType.Relu
    dr = mybir.MatmulPerfMode.DoubleRow

    with tile.TileContext(nc) as tc:
        with (
            tc.tile_pool(name="xp", bufs=1) as xp,
            tc.tile_pool(name="wp", bufs=4) as wp,
            tc.tile_pool(name="op", bufs=3) as op,
            tc.tile_pool(name="bp", bufs=1) as bp,
            tc.tile_pool(name="pp", bufs=2, space="PSUM") as pp,
        ):
            # First weight tile split in two so the first matmul group can
            # start as soon as the first half lands. Weight stream rides the
            # sync ring alone; slot waits (wp bufs) pace it behind the PE.
            wtiles = {}
            wtiles[0] = wp.tile([128, KT2, 2, 128], f8, name="wtile", tag="w")
            for q in range(2):
                nc.sync.dma_start(
                    out=wtiles[0][:, q * 4 : (q + 1) * 4, :, :],
                    in_=wt_d[0, :, q * 4 : (q + 1) * 4, :, :],
                )

            btile = bp.tile([128, OT], f32, name="btile")
            nc.scalar.dma_start(out=btile, in_=b_d[:, :])

            # x8 on the gpsimd ring in 4 slabs (first slab gates the PE).
            x8all = xp.tile([128, KT2, 2, np_tokens], f8, name="x8all")
            for s in range(0, KT2, 2):
                nc.gpsimd.dma_start(
                    out=x8all[:, s : s + 2, :, :],
                    in_=x8_d[:, s : s + 2, :, :],
                )

            # f16 residual on the scalar ring in 4 slabs; slab dt covers
            # o-tiles 4dt..4dt+3, needed only at activation time.
            xr_r = xr_d.rearrange("(t p) n -> p t n", p=128)
            xrall = xp.tile([128, OT, np_tokens], f16, name="xrall")
            for s in range(0, OT, 4):
                nc.scalar.dma_start(
                    out=xrall[:, s : s + 4, :],
                    in_=xr_r[:, s : s + 4, :],
                )

            # Whole weight stream up front on the sync ring.
            for ot in range(1, OT):
                wtiles[ot] = wp.tile([128, KT2, 2, 128], f8, name="wtile", tag="w")
                nc.sync.dma_start(out=wtiles[ot], in_=wt_d[ot])

            for ot in range(OT):
                wtile = wtiles[ot]
                otile = op.tile([128, np_tokens], f16, name="otile", tag="o")
                psums = [
                    pp.tile([128, ch], f32, name=f"ps{ci}", tag=f"ps{ci}")
                    for ci, (_, ch) in enumerate(chunks)
                ]
                for kt2 in range(KT2):
                    lhsT = wtile[:, kt2, :, :]
                    for ci, (n0, ch) in enumerate(chunks):
                        nc.tensor.matmul(
                            psums[ci],
                            lhsT,
                            x8all[:, kt2, :, n0 : n0 + ch],
                            start=(kt2 == 0),
                            stop=(kt2 == KT2 - 1),
                            perf_mode=dr,
                        )
                for ci, (n0, ch) in enumerate(chunks):
                    nc.scalar.activation(
                        otile[:, n0 : n0 + ch],
                        psums[ci],
                        relu,
                        bias=btile[:, ot : ot + 1],
                        scale=1.0 / W_SCALE,
                    )
                    nc.vector.tensor_add(
                        otile[:, n0 : n0 + ch],
                        otile[:, n0 : n0 + ch],
                        xrall[:, ot, n0 : n0 + ch],
                    )
                    if ot == OT - 1:
                        # last tile: store per-chunk to shrink the tail
                        nc.gpsimd.dma_start(
                            out=out_d[ot, :, n0 : n0 + ch],
                            in_=otile[:, n0 : n0 + ch],
                        )
                if ot < OT - 1:
                    nc.gpsimd.dma_start(out=out_d[ot], in_=otile)
    nc.compile()
    return nc


def kernel(x, cam_pred_ids, W, b, _want_results=False):
    x = np.ascontiguousarray(np.asarray(x), dtype=np.float32)
    W = np.asarray(W, dtype=np.float32)
    b = np.asarray(b, dtype=np.float32)
    ids = np.asarray(cam_pred_ids).astype(np.int64)
    f8 = ml_dtypes.float8_e4m3

    counts = np.bincount(ids, minlength=NUM_EXPERTS)
    order = np.argsort(ids, kind="stable")
    np_tokens = max(512, int(counts.max()))

    # per-expert padded token index lists (pad with token 0; discarded later)
    starts = np.zeros(NUM_EXPERTS + 1, dtype=np.int64)
    np.cumsum(counts, out=starts[1:])
    idx = np.zeros((NUM_EXPERTS, np_tokens), dtype=np.int64)
    for e in range(NUM_EXPERTS):
        idx[e, : counts[e]] = order[starts[e] : starts[e + 1]]

    in_maps = []
    for e in range(NUM_EXPERTS):
        xg = x[idx[e]]  # [Np, DIM]
        xt = np.ascontiguousarray(xg.T)  # [DIM, Np]
        # x8[k_lo, kt2, i, n]
        x8 = np.ascontiguousarray(
            xt.reshape(KT2, 2, 128, np_tokens).transpose(2, 0, 1, 3), dtype=f8
        )
        # wt[ot, k_lo, kt2, i, o] = 32*W[e][128*ot+o, 256*kt2+128*i+k_lo]
        wt = np.ascontiguousarray(
            (W_SCALE * W[e])
            .reshape(OT, 128, KT2, 2, 128)
            .transpose(0, 4, 2, 3, 1),
            dtype=f8,
        )
        m = {
            "wt": wt,
            "x8": x8,
            "xr": np.ascontiguousarray(xt, dtype=np.float16),
            "b": np.ascontiguousarray(b[e].reshape(OT, 128).T),
        }
        in_maps.append(m)

    nc = _build_nc(np_tokens)
    res = run_bass_kernel_spmd(
        nc,
        in_maps,
        core_ids=list(range(NUM_EXPERTS)),
        trace=bool(int(os.environ.get("BASS_MOE_TRACE", "0"))),
    )

    out = np.empty_like(x)
    for e in range(NUM_EXPERTS):
        oute = res.results[e]["out"]  # [OT, 128, Np] f16
        valid = idx[e, : counts[e]]
        out[valid] = (
            oute.reshape(DIM, np_tokens).T[: counts[e]].astype(np.float32)
        )
    if _want_results:
        return out, res
    return out


# revision 5
# speedup vs baseline: 1.7951x; 1.0234x over previous
"""MoE routing kernel for Trainium2 (8 NeuronCores, expert-parallel, fp8).

out[i] = x[i] + relu(x[i] @ W[e].T + b[e]),  e = cam_pred_ids[i]

Strategy: route tokens by expert on the host, so core e computes ONLY
expert e's tokens with ONLY W[e]. The matmul runs in fp8 e4m3 with
DoubleRow perf mode (two 128-deep K planes contracted per instruction,
2x the f16 PE rate), with W pre-scaled by 32 on the host to stay in
e4m3 normal range; the activation step rescales by 1/32 and applies
bias+relu. The residual is added from a separate f16 copy of x and the
output is stored as f16 (error is dominated by the fp8 weights;
measured end-to-end rel err ~1.4e-2, within the 2e-2 budget).

Layouts are pre-transposed on the host so every DMA moves >=2KB
contiguous per partition. Ring plan: sync = weight stream + output
stores (interleaved, emitted in-loop); gpsimd = x8 loads only (its
expensive dge_drain then fires early, hidden under the PE window);
scalar = bias + f16 residual + activations; vector = residual adds.
"""

import os
import numpy as np
import ml_dtypes

import concourse.bass as bass
from concourse import bacc
import concourse.mybir as mybir
import concourse.tile as tile
from concourse.bass_utils import run_bass_kernel_spmd

NUM_EXPERTS = 8
DIM = 2048
KT2 = DIM // 256  # 8 double-row k groups (256 contraction each)
OT = DIM // 128  # 16 o-tiles

W_SCALE = 32.0
MODE = "fp8dr"  # fp8 e4m3 DoubleRow
WPREFETCH = 4  # weight tiles requested ahead of the consuming o-tile


def _chunks(np_tokens: int) -> list[tuple[int, int]]:
    """Split the free dim into matmul chunks of <=512 (PSUM bank limit),
    each >=256 when possible."""
    out = []
    pos = 0
    rem = np_tokens
    while rem > 0:
        if rem > 512 + 256:
            take = 512
        elif rem > 512:
            take = (rem + 1) // 2  # two chunks, both >=256
        else:
            take = rem
        out.append((pos, take))
        pos += take
        rem -= take
    return out


def _build_nc(np_tokens: int):
    f32 = mybir.dt.float32
    f16 = mybir.dt.float16
    f8 = mybir.dt.float8e4

    nc = bacc.Bacc()
    # wt[ot, k_lo, kt2, i, o] = 32*W[e][128*ot+o, 256*kt2+128*i+k_lo]
    wt_d = nc.declare_dram_parameter("wt", [OT, 128, KT2, 2, 128], f8, isOutput=False)
    # x8[k_lo, kt2, i, n] = fp8(x[n, 256*kt2+128*i+k_lo])
    x8_d = nc.declare_dram_parameter("x8", [128, KT2, 2, np_tokens], f8, isOutput=False)
    # xr[(dt p), n] = f16(x[n, 128*dt+p])   (residual, d on partitions)
    xr_d = nc.declare_dram_parameter("xr", [DIM, np_tokens], f16, isOutput=False)
    b_d = nc.declare_dram_parameter("b", [128, OT], f32, isOutput=False)
    out_d = nc.declare_dram_parameter("out", [OT, 128, np_tokens], f16, isOutput=True)

    chunks = _chunks(np_tokens)
    relu = mybir.ActivationFunctionType.Relu
    dr = mybir.MatmulPerfMode.DoubleRow

    with tile.TileContext(nc) as tc:
        with (
            tc.tile_pool(name="xp", bufs=1) as xp,
            tc.tile_pool(name="wp", bufs=WPREFETCH + 1) as wp,
            tc.tile_pool(name="op", bufs=3) as op,
            tc.tile_pool(name="bp", bufs=1) as bp,
            tc.tile_pool(name="pp", bufs=2, space="PSUM") as pp,
        ):
            # First weight tile split in two so the first matmul group can
            # start as soon as the first half lands.
            wtiles = {}
            wtiles[0] = wp.tile([128, KT2, 2, 128], f8, name="wtile", tag="w")
            for q in range(2):
                nc.sync.dma_start(
                    out=wtiles[0][:, q * 4 : (q + 1) * 4, :, :],
                    in_=wt_d[0, :, q * 4 : (q + 1) * 4, :, :],
                )

            btile = bp.tile([128, OT], f32, name="btile")
            nc.scalar.dma_start(out=btile, in_=b_d[:, :])

            # x8 on the gpsimd ring; small first slabs so the PE can start
            # as early as possible, then bigger ones.
            x8all = xp.tile([128, KT2, 2, np_tokens], f8, name="x8all")
            s = 0
            for w in (1, 1, 2, 2, 2):
                nc.gpsimd.dma_start(
                    out=x8all[:, s : s + w, :, :],
                    in_=x8_d[:, s : s + w, :, :],
                )
                s += w

            # f16 residual on the scalar ring in 4 slabs; slab dt covers
            # o-tiles 4dt..4dt+3, needed only at activation time.
            xr_r = xr_d.rearrange("(t p) n -> p t n", p=128)
            xrall = xp.tile([128, OT, np_tokens], f16, name="xrall")
            for s in range(0, OT, 4):
                nc.scalar.dma_start(
                    out=xrall[:, s : s + 4, :],
                    in_=xr_r[:, s : s + 4, :],
                )

            # Prefetch the next few weight tiles; the rest are emitted
            # inside the o-tile loop so the sync ring interleaves them
            # with the output stores (FIFO per ring).
            for ot in range(1, WPREFETCH):
                wtiles[ot] = wp.tile([128, KT2, 2, 128], f8, name="wtile", tag="w")
                nc.sync.dma_start(out=wtiles[ot], in_=wt_d[ot])

            for ot in range(OT):
                wtile = wtiles[ot]
                otile = op.tile([128, np_tokens], f16, name="otile", tag="o")
                psums = [
                    pp.tile([128, ch], f32, name=f"ps{ci}", tag=f"ps{ci}")
                    for ci, (_, ch) in enumerate(chunks)
                ]
                last = ot == OT - 1
                if last:
                    # chunk-outer so chunk 0's act/add/store overlap the
                    # remaining chunks' matmuls (shrinks the tail)
                    for ci, (n0, ch) in enumerate(chunks):
                        for kt2 in range(KT2):
                            nc.tensor.matmul(
                                psums[ci],
                                wtile[:, kt2, :, :],
                                x8all[:, kt2, :, n0 : n0 + ch],
                                start=(kt2 == 0),
                                stop=(kt2 == KT2 - 1),
                                perf_mode=dr,
                            )
                else:
                    for kt2 in range(KT2):
                        lhsT = wtile[:, kt2, :, :]
                        for ci, (n0, ch) in enumerate(chunks):
                            nc.tensor.matmul(
                                psums[ci],
                                lhsT,
                                x8all[:, kt2, :, n0 : n0 + ch],
                                start=(kt2 == 0),
                                stop=(kt2 == KT2 - 1),
                                perf_mode=dr,
                            )
                for ci, (n0, ch) in enumerate(chunks):
                    nc.scalar.activation(
                        otile[:, n0 : n0 + ch],
                        psums[ci],
                        relu,
                        bias=btile[:, ot : ot + 1],
                        scale=1.0 / W_SCALE,
                    )
                    nc.vector.tensor_add(
                        otile[:, n0 : n0 + ch],
                        otile[:, n0 : n0 + ch],
                        xrall[:, ot, n0 : n0 + ch],
                    )
                    if last:
                        # per-chunk stores on alternating rings to
                        # parallelize the tail
                        eng = nc.scalar if ci == 1 else nc.sync
                        eng.dma_start(
                            out=out_d[ot, :, n0 : n0 + ch],
                            in_=otile[:, n0 : n0 + ch],
                        )
                if not last:
                    nc.sync.dma_start(out=out_d[ot], in_=otile)
                nxt = ot + WPREFETCH
                if nxt < OT:
                    wtiles[nxt] = wp.tile(
                        [128, KT2, 2, 128], f8, name="wtile", tag="w"
                    )
                    nc.sync.dma_start(out=wtiles[nxt], in_=wt_d[nxt])
    nc.compile()
    return nc


def kernel(x, cam_pred_ids, W, b, _want_results=False):
    x = np.ascontiguousarray(np.asarray(x), dtype=np.float32)
    W = np.asarray(W, dtype=np.float32)
    b = np.asarray(b, dtype=np.float32)
    ids = np.asarray(cam_pred_ids).astype(np.int64)
    f8 = ml_dtypes.float8_e4m3

    counts = np.bincount(ids, minlength=NUM_EXPERTS)
    order = np.argsort(ids, kind="stable")
    np_tokens = max(512, int(counts.max()))

    # per-expert padded token index lists (pad with token 0; discarded later)
    starts = np.zeros(NUM_EXPERTS + 1, dtype=np.int64)
    np.cumsum(counts, out=starts[1:])
    idx = np.zeros((NUM_EXPERTS, np_tokens), dtype=np.int64)
    for e in range(NUM_EXPERTS):
        idx[e, : counts[e]] = order[starts[e] : starts[e + 1]]

    in_maps = []
    for e in range(NUM_EXPERTS):
        xg = x[idx[e]]  # [Np, DIM]
        xt = np.ascontiguousarray(xg.T)  # [DIM, Np]
        # x8[k_lo, kt2, i, n]
        x8 = np.ascontiguousarray(
            xt.reshape(KT2, 2, 128, np_tokens).transpose(2, 0, 1, 3), dtype=f8
        )
        # wt[ot, k_lo, kt2, i, o] = 32*W[e][128*ot+o, 256*kt2+128*i+k_lo]
        wt = np.ascontiguousarray(
            (W_SCALE * W[e])
            .reshape(OT, 128, KT2, 2, 128)
            .transpose(0, 4, 2, 3, 1),
            dtype=f8,
        )
        m = {
            "wt": wt,
            "x8": x8,
            "xr": np.ascontiguousarray(xt, dtype=np.float16),
            "b": np.ascontiguousarray(b[e].reshape(OT, 128).T),
        }
        in_maps.append(m)

    nc = _build_nc(np_tokens)
    res = run_bass_kernel_spmd(
        nc,
        in_maps,
        core_ids=list(range(NUM_EXPERTS)),
        trace=bool(int(os.environ.get("BASS_MOE_TRACE", "0"))),
    )

    out = np.empty_like(x)
    for e in range(NUM_EXPERTS):
        oute = res.results[e]["out"]  # [OT, 128, Np] f16
        valid = idx[e, : counts[e]]
        out[valid] = (
            oute.reshape(DIM, np_tokens).T[: counts[e]].astype(np.float32)
        )
    if _want_results:
        return out, res
    return out
